# revision 5
# baseline (speedup 1.0000x reference)
"""Trainium2 Bass kernel for MetaGNNNoEdgeAttr (GNN message passing).

Strategy (8 NeuronCores, SPMD):
  - Undirected+self-loop graph; bipartite => edges into half-A nodes have
    src in half B and vice versa.  Nodes are packed into (core, block,
    slot): 4 cores per half, degree-balanced so every block has at most
    SUBT*128 incoming directed edges -> uniform SPMD schedule.
  - Algebra: kqv/W1 folded on host: Kp = x @ Wkp, Qp = x @ Wqp, V = x @ Wv.
    Signed W2 is folded into the Kp/Qp table columns (permuted so
    nonneg-W2 features come first):  w2_j*relu(z_j) = max(w2_j z_j, 0) for
    w2_j>=0 else min(w2_j z_j, 0), so the per-edge logit is a plain sum of
    a max-part and a min-part -- no separate relu/mult passes.
  - fp8 tables (gamma keeps values <= ~120; device e4m3 saturates ~240),
    V scaled by SV=8 (ones_col=SV cancels it in the softmax ratio).
    Kp|V fused in one [SLOTS, 512] fp8 table -> one AllGather per
    half-group and a single 512B-per-edge gather.
  - Phase A pass 1: partner Kp|V table -> fp8 -> group AllGather.  Pass 2:
    own Qp|V resident (fp8); self-loop logits from the Wkpq matmul via
    one TS-max + signed-mask PE column sum.
  - Edge phase per gather batch (GB=4096 slots): one fused KpV gather;
    per subtile the dst-side Qp is computed ON the PE (host-precomputed
    one-hot ohTq matmul from resident QV) and Kp added via an identity
    matmul into the same PSUM -- no Qp gather at all; PSUM -> z copies
    alternate ACT/DVE; TS max+min (4x) then a pairwise TT tree sum (2x)
    replaces the 1x tensor_reduce; exp on ACT; aggregation + softmax
    denominator via one-hot PE matmuls; Wout per head; normalization via
    ACT scale; relu + residual; batched output stores.
  - Cross-repeat software pipelining: phase A of iteration k+1 (pass 1
    chunks, the AllGather, pass 2 chunks) is emitted interleaved between
    the edge batches of iteration k, with ping-pong DRAM tables and
    double-buffered residents, so the collective and the projection
    matmuls hide behind the edge phase in steady state.
"""

import os
import sys
import math
import numpy as np

for _p in ("/opt/trn_rl_repo", "/root/.axon_site/_ro/trn_rl_repo"):
    if os.path.isdir(_p) and _p not in sys.path:
        sys.path.insert(0, _p)

import ml_dtypes  # noqa: E402

BF16 = ml_dtypes.bfloat16
FP8 = ml_dtypes.float8_e4m3

SK = 256.0  # Kp/Qp fp8 scale (folded into Wkp/Wqp; W2 divided by it)
SV = 8.0    # V fp8 scale (ones_col = SV cancels it in the softmax ratio)

# ---------------------------------------------------------------- host prep


def _pack_half(deg_half, n_bins, cap_edges):
    """Pack nodes (by local id within half) into n_bins bins of <=128 nodes,
    each bin with sum(deg) <= cap_edges. Returns [n_bins] lists of local ids."""
    nh = deg_half.shape[0]
    order = np.argsort(-deg_half, kind="stable")
    per_bin = (nh + n_bins - 1) // n_bins
    assert per_bin <= 128
    bins = [[] for _ in range(n_bins)]
    loads = np.zeros(n_bins, dtype=np.int64)
    # deal degree-sorted nodes in rounds: heaviest of each round to the
    # currently lightest bins
    for r in range(per_bin):
        chunk = order[r * n_bins : (r + 1) * n_bins]
        target = np.argsort(loads, kind="stable")
        for k, nd in enumerate(chunk):
            b = int(target[k])
            bins[b].append(int(nd))
            loads[b] += deg_half[nd]
    # fixup: move low-degree nodes from overloaded to underloaded bins
    sizes = np.array([len(x) for x in bins])
    for _ in range(20000):
        hi = int(np.argmax(loads))
        if loads[hi] <= cap_edges:
            break
        lo = int(np.argmin(loads + (sizes >= 128) * 10**9))
        bl = bins[hi]
        j = int(np.argmin(deg_half[bl]))
        nd = bl.pop(j)
        bins[lo].append(nd)
        loads[hi] -= deg_half[nd]
        loads[lo] += deg_half[nd]
        sizes[hi] -= 1
        sizes[lo] += 1
    assert loads.max() <= cap_edges, (loads.max(), cap_edges)
    assert max(len(b) for b in bins) <= 128
    return bins


def prep(x, edge_index, Wkqv, bkqv, W1, b1, W2, b2, Wout, bout, n_cores=8):
    """All host-side preprocessing. Returns (meta, per_core_inputs, const_inputs,
    scatter info for output assembly)."""
    N, EMB = x.shape
    H, hd = 2, EMB // 2
    HALF = N // 2
    GROUP = n_cores // 2

    ei = np.asarray(edge_index).astype(np.int64)
    src = np.concatenate([ei[0], ei[1]])
    dst = np.concatenate([ei[1], ei[0]])
    assert src.min() >= 0 and src.max() < N

    deg = np.bincount(dst, minlength=N)

    # ---- choose BLOCKS / SUBT: minimize total subtiles T = BLOCKS * SUBT
    B0 = max(1, (HALF + GROUP * 128 - 1) // (GROUP * 128))
    best = None
    for BLOCKS in range(B0, B0 + 4):
        n_bins = GROUP * BLOCKS
        load = max(deg[:HALF].sum(), deg[HALF:].sum()) / n_bins
        SUBT = max(1, int(math.ceil(load / 128)))
        for _try in range(3):
            cap = SUBT * 128
            try:
                bins_a = _pack_half(deg[:HALF], n_bins, cap)
                bins_b = _pack_half(deg[HALF:], n_bins, cap)
            except AssertionError:
                SUBT += 1
                continue
            if best is None or BLOCKS * SUBT < best[0] * best[1]:
                best = (BLOCKS, SUBT, bins_a, bins_b)
            break
    if best is None:
        raise RuntimeError("packing failed")
    BLOCKS, SUBT, bins_a, bins_b = best
    n_bins = GROUP * BLOCKS
    SLOTS = BLOCKS * 128
    T = BLOCKS * SUBT  # subtiles per core

    # ---- node -> (core, block, slot); core 0..GROUP-1 own half A
    core_of = np.full(N, -1, np.int32)
    block_of = np.full(N, -1, np.int32)
    slot_of = np.full(N, -1, np.int32)
    node_of = np.full((n_cores, SLOTS), -1, np.int64)
    for half, bins in ((0, bins_a), (1, bins_b)):
        for i, bl in enumerate(bins):
            c = half * GROUP + i % GROUP
            b = i // GROUP
            for s, nd_local in enumerate(sorted(bl)):
                nd = nd_local + half * HALF
                core_of[nd] = c
                block_of[nd] = b
                slot_of[nd] = s
                node_of[c, b * 128 + s] = nd
    assert (core_of >= 0).all()

    # opposite-half row of a node (position in the group AllGather output):
    # contribution order within each group AG is group position 0..GROUP-1.
    H1 = SLOTS
    own_row = block_of * 128 + slot_of
    gp = (core_of % GROUP).astype(np.int64)
    opp_row = gp * SLOTS + own_row

    # ---- per-core edge slot assignment
    # gather batch = BPB whole blocks (~2048 edge slots)
    BPB = max(1, int(os.environ.get("GBT", "4096")) // (SUBT * 128))
    GB = BPB * SUBT * 128  # edge slots per gather batch
    NB = (BLOCKS + BPB - 1) // BPB
    per_core = []
    ecore = core_of[dst]
    for c in range(n_cores):
        m = ecore == c
        es, ed = src[m], dst[m]
        eb = block_of[ed]
        order = np.argsort(eb, kind="stable")
        es, ed, eb = es[order], ed[order], eb[order]
        counts = np.bincount(eb, minlength=BLOCKS)
        assert counts.max() <= SUBT * 128
        kp_idx = np.zeros(T * 128, np.int16)
        dstrel = np.full(T * 128, -1.0, np.float32)
        pos = 0
        for b in range(BLOCKS):
            n_b = counts[b]
            sl = slice(b * SUBT * 128, b * SUBT * 128 + n_b)
            seg = slice(pos, pos + n_b)
            kp_idx[sl] = opp_row[es[seg]].astype(np.int16)
            dstrel[sl] = slot_of[ed[seg]].astype(np.float32)
            pos += n_b
        assert opp_row[es].max() < 32768 and GROUP * SLOTS < 32768

        # wrap indices: unwrapped[i] = arr[i%16, i//16], replicated to 128 parts
        def wrap(a):
            out = np.zeros((NB, 128, GB // 16), np.int16)
            for g in range(NB):
                seg = a[g * GB : (g + 1) * GB]
                w = np.zeros((16, GB // 16), np.int16)
                n = len(seg)
                ii = np.arange(n)
                w[ii % 16, ii // 16] = seg
                out[g] = np.tile(w, (8, 1))
            return out

        xo = np.zeros((SLOTS, EMB), np.float32)
        vmask = node_of[c] >= 0
        xo[vmask] = x[node_of[c][vmask]]
        p = (c + GROUP) % n_cores  # partner core (opposite half)
        xp = np.zeros((SLOTS, EMB), np.float32)
        vmaskp = node_of[p] >= 0
        xp[vmaskp] = x[node_of[p][vmaskp]]

        def xT(a):  # [SLOTS, EMB] -> [128, EMB//128, SLOTS] fp8
            return np.ascontiguousarray(
                a.reshape(SLOTS, EMB // 128, 128).transpose(2, 1, 0)
            ).astype(FP8)

        dstrel_te = dstrel.reshape(T, 128)  # [t, e]
        ohTq = np.ascontiguousarray(
            (dstrel_te[None, :, :] == np.arange(128, dtype=np.float32)[:, None, None])
        ).astype(FP8)  # [d, t, e]
        per_core.append(
            dict(
                xTp=xT(xp),
                xTo=xT(xo),
                x_own=np.ascontiguousarray(
                    xo.reshape(BLOCKS, 128, EMB).transpose(1, 0, 2)
                ).astype(BF16),
                kp_ix=wrap(kp_idx),
                ohTq=ohTq,
                dstrel=np.ascontiguousarray(dstrel_te.T),
            )
        )

    # ---- folded weights
    W1k, W1q = W1[:hd].astype(np.float64), W1[hd:].astype(np.float64)
    bd = lambda A: np.block(
        [[A, np.zeros_like(A)], [np.zeros_like(A), A]]
    )  # [256,256]
    Wq = Wkqv[:, :EMB].astype(np.float64)
    Wk = Wkqv[:, EMB : 2 * EMB].astype(np.float64) / math.sqrt(hd)
    Wv = Wkqv[:, 2 * EMB :].astype(np.float64)
    Wkp = Wk @ bd(W1k)
    Wqp = Wq @ bd(W1q)
    Wvs = Wv * SV

    # Fold signed W2 into the Kp/Qp table columns and permute features so
    # nonneg-W2 features come first per head:
    #   w2_j * relu(z_j) = max(w2_j*z_j, 0)  if w2_j >= 0
    #                    = min(w2_j*z_j, 0)  if w2_j <  0
    # so per-edge logits become a plain segmented SUM of max/min regions.
    w2 = W2[:hd].astype(np.float64).reshape(-1)  # [128]
    perm = np.concatenate(
        [np.where(w2 >= 0)[0], np.where(w2 < 0)[0]]
    )
    P0 = int((w2 >= 0).sum())
    fold = w2[perm]  # signed fold factor per permuted feature
    idx2 = np.concatenate([perm, 128 + perm])  # both heads
    fold2 = np.concatenate([fold, fold])
    Wkp2 = Wkp[:, idx2] * fold2[None, :]
    Wqp2 = Wqp[:, idx2] * fold2[None, :]
    # fp8 tables: global scale keeps on-device values within e4m3 range
    est_k = np.abs(np.asarray(x, np.float64) @ Wkp2).max()
    est_q = np.abs(np.asarray(x, np.float64) @ Wqp2).max()
    gamma = 120.0 / max(est_k, est_q, 1e-30)
    Wkp2 *= gamma
    Wqp2 *= gamma
    # self-loop table: fold |w2| so a_self = sum_j sign_j * max(z_j, 0)
    s2 = np.where(fold2 >= 0, 1.0, -1.0)
    Wkpq = (Wkp2 + Wqp2) * s2[None, :]

    if not (
        np.all(bkqv == 0) and np.all(b1 == 0) and np.all(bout == 0)
    ):
        raise NotImplementedError("nonzero bkqv/b1/bout not supported")

    # SBUF layout [128, 2, F]: [p, c, e] = W[c*128+p, e]
    def chunk(W, F):
        return np.ascontiguousarray(
            W.astype(np.float32).reshape(2, 128, F).transpose(1, 0, 2)
        ).astype(BF16)

    Wkv = np.concatenate([Wkp2, Wvs], axis=1)   # [256, 512] -> kp | v
    Wqv = np.concatenate([Wqp2, Wvs], axis=1)   # [256, 512] -> qp | vown
    CW = GB // 128  # subtile-columns per full gather batch
    consts = dict(
        Wkv_c=chunk(Wkv, 512),
        Wqv_c=chunk(Wqv, 512),
        Wkpq_c=chunk(Wkpq, EMB),
        Wout_c=chunk(Wout.astype(np.float64), EMB),
        iota_bc=np.tile(np.arange(128, dtype=BF16)[None, :], (128, 1)),
        ident_c=np.eye(128, dtype=FP8),
        iota_col=np.arange(128, dtype=np.float32)[:, None],
        ones_col=np.full((128, 1), SV, BF16),
        spm_col=np.where(np.arange(128)[:, None] < P0, 1.0, -1.0).astype(BF16),
    )
    meta = dict(
        N=N,
        EMB=EMB,
        HALF=HALF,
        GROUP=GROUP,
        BLOCKS=BLOCKS,
        SLOTS=SLOTS,
        SUBT=SUBT,
        T=T,
        GB=GB,
        NB=NB,
        BPB=BPB,
        CW=CW,
        H1=H1,
        n_cores=n_cores,
        b2=float(np.asarray(b2).reshape(-1)[0]),
        P0=P0,
        invg=float(1.0 / gamma),
    )
    return meta, per_core, consts, node_of


# ------------------------------------------------------------- bass program


def build_program(meta, repeats=1):
    import concourse.bass as bass
    import concourse.tile as tile
    from concourse import bacc, mybir

    AF = mybir.ActivationFunctionType
    ALU = mybir.AluOpType
    BF = mybir.dt.bfloat16
    F32 = mybir.dt.float32
    F8 = mybir.dt.float8e4
    I16 = mybir.dt.int16

    EMB = meta["EMB"]
    BLOCKS, SLOTS, SUBT, T = (
        meta["BLOCKS"],
        meta["SLOTS"],
        meta["SUBT"],
        meta["T"],
    )
    GB, NB, GROUP, BPB, CW = (
        meta["GB"], meta["NB"], meta["GROUP"], meta["BPB"], meta["CW"],
    )
    H1 = meta["H1"]
    n_cores = meta["n_cores"]
    b2f = meta["b2"]
    P0 = meta["P0"]
    invg = meta["invg"]
    NB4 = (BLOCKS + 3) // 4  # phase-A 4-block chunks
    AG1_C4 = H1 // 512  # pass-1 chunks contributing to AllGather chunk 1

    nc = bacc.Bacc(
        "TRN2", target_bir_lowering=False, debug=False, num_devices=n_cores,
        num_swdge_queues=2 if os.environ.get("SWQ2") else 1,
    )

    dram = lambda name, shape, dt: nc.dram_tensor(
        name, shape, dt, kind="ExternalInput"
    )
    xTp_d = dram("xTp", [128, 2, SLOTS], F8)
    xTo_d = dram("xTo", [128, 2, SLOTS], F8)
    x_own_d = dram("x_own", [128, BLOCKS, EMB], BF)
    kp_ix_d = dram("kp_ix", [NB, 128, GB // 16], I16)
    ohTq_d = dram("ohTq", [128, T, 128], F8)
    dstrel_d = dram("dstrel", [128, T], F32)
    Wkv_d = dram("Wkv_c", [128, 2, 512], BF)
    Wqv_d = dram("Wqv_c", [128, 2, 512], BF)
    Wkpq_d = dram("Wkpq_c", [128, 2, EMB], BF)
    Wout_d = dram("Wout_c", [128, 2, EMB], BF)
    iota_bc_d = dram("iota_bc", [128, 128], BF)
    ident_d = dram("ident_c", [128, 128], F8)
    iota_col_d = dram("iota_col", [128, 1], F32)
    ones_col_d = dram("ones_col", [128, 1], BF)
    spm_col_d = dram("spm_col", [128, 1], BF)
    out_d = nc.dram_tensor("out", [128, BLOCKS, EMB], BF, kind="ExternalOutput")

    groups2 = [
        list(range(GROUP)),
        list(range(GROUP, n_cores)),
    ]

    with tile.TileContext(nc) as tc:
        # internal DRAM (collective buffers + local qp table)
        _frees = []

        def _dram_tile(shape, name, addr_space="Local"):
            t, free = tc.tile(
                shape,
                F8,
                space=bass.MemorySpace.DRAM,
                addr_space=addr_space,
                name=name,
            )
            _frees.append(free)
            return t

        kpv_agin = [_dram_tile([SLOTS, 512], f"kpv_agin{i}") for i in range(2)]
        kpv_opp = [
            _dram_tile([GROUP * SLOTS, 512], f"kpv_opp{i}", "Shared")
            for i in range(2)
        ]

        with (
            tc.tile_pool(name="res", bufs=1) as res,
            tc.tile_pool(name="dbl", bufs=2) as dbl,
            tc.tile_pool(name="pA", bufs=2) as pA,
            tc.tile_pool(name="psA", bufs=1, space="PSUM") as psA,
            tc.tile_pool(name="pB", bufs=2) as pB,
            tc.tile_pool(name="psB", bufs=1, space="PSUM") as psB,
            tc.tile_pool(name="psS", bufs=1, space="PSUM") as psS,
            tc.tile_pool(name="pG", bufs=2) as pG,
            tc.tile_pool(name="pW", bufs=2 if BPB <= 4 else 1) as pW,
            tc.tile_pool(name="pE", bufs=2) as pE,
            tc.tile_pool(name="psE", bufs=2, space="PSUM") as psE,
            tc.tile_pool(name="psQ", bufs=2, space="PSUM") as psQ,
        ):
            x_res = res.tile([128, BLOCKS, EMB], BF)
            dstrel_sb = res.tile([128, T], F32)
            iota_bc = res.tile([128, 128], BF)
            ident_sb = res.tile([128, 128], F8)
            iota_col = res.tile([128, 1], F32)
            ones_col = res.tile([128, 1], BF)
            spm_col = res.tile([128, 1], BF)
            Wkv_sb = res.tile([128, 2, 512], BF)
            Wqv_sb = res.tile([128, 2, 512], BF)
            Wkpq_sb = res.tile([128, 2, EMB], BF)
            Wout_sb = res.tile([128, 2, EMB], BF)
            kpix_sb = res.tile([128, NB, GB // 16], I16)
            ohTq_sb = res.tile([128, T, 128], F8)

            # repeat-invariant loads (inputs/weights stay resident across
            # iterations in steady state)
            nc.sync.dma_start(dstrel_sb[:], dstrel_d[:])
            nc.sync.dma_start(iota_bc[:], iota_bc_d[:])
            nc.sync.dma_start(ident_sb[:], ident_d[:])
            nc.sync.dma_start(iota_col[:], iota_col_d[:])
            nc.sync.dma_start(ones_col[:], ones_col_d[:])
            nc.sync.dma_start(spm_col[:], spm_col_d[:])
            nc.sync.dma_start(x_res[:], x_own_d[:])
            nc.sync.dma_start(
                kpix_sb[:], kp_ix_d[:].rearrange("g p w -> p g w")
            )
            nc.sync.dma_start(ohTq_sb[:], ohTq_d[:])
            for w_sb, w_d in (
                (Wkv_sb, Wkv_d),
                (Wqv_sb, Wqv_d),
                (Wkpq_sb, Wkpq_d),
                (Wout_sb, Wout_d),
            ):
                nc.sync.dma_start(w_sb[:], w_d[:])

            ABL_GATHER = bool(os.environ.get("ABL_GATHER"))
            ABL_EMATH = bool(os.environ.get("ABL_EMATH"))
            ABL_AGG = bool(os.environ.get("ABL_AGG"))
            if ABL_GATHER:
                KpV_dum = res.tile([128, GB // 128, 512], F8)
                nc.vector.memset(KpV_dum[:], 0)
            if ABL_EMATH:
                ex_dum = res.tile([128, GB // 128, 2], F32)
                nc.vector.memset(ex_dum[:], 1.0)
            if ABL_AGG:
                ost_dum = res.tile([128, BPB, EMB], BF)
                nc.vector.memset(ost_dum[:], 0)

            gat_count = [0]  # global gather counter (SWQ2 queue parity)

            # per-iteration rotating state (parity r = rep % 2)
            QV_t = {}    # rep -> QV resident tile (Qp|V fp8)
            exS_t = {}   # rep -> exp(a_self) tile
            aself_t = {}  # rep -> PSUM logits tile

            def pass1_chunk(rep, c4):
                """partner Kp|V table chunk -> kpv_agin[rep % 2]"""
                nb4 = min(BLOCKS - c4 * 4, 4)
                w = nb4 * 128
                sl = slice(c4 * 512, c4 * 512 + w)
                xtp = pA.tile([128, 2, 512], F8, tag="xtp")
                nc.sync.dma_start(xtp[:, :, :w], xTp_d[:, :, sl])
                st1 = pA.tile([128, 4, 512], F8, tag="st1")
                for bi in range(nb4):
                    bsl = slice(bi * 128, (bi + 1) * 128)
                    ps1 = psA.tile([128, 512], F32, tag="ps1")
                    nc.tensor.matmul(
                        ps1[:], xtp[:, 0, bsl], Wkv_sb[:, 0, :],
                        start=True, stop=False,
                    )
                    nc.tensor.matmul(
                        ps1[:], xtp[:, 1, bsl], Wkv_sb[:, 1, :],
                        start=False, stop=True,
                    )
                    nc.scalar.activation(st1[:, bi, :], ps1[:], AF.Copy)
                nc.sync.dma_start(
                    kpv_agin[rep % 2][sl, :].rearrange(
                        "(b p) f -> p b f", p=128
                    ),
                    st1[:, :nb4, :],
                )

            def collective(rep):
                if os.environ.get("ABLATE_COLL"):
                    nc.sync.dma_start(
                        kpv_opp[rep % 2][0:SLOTS, :], kpv_agin[rep % 2][:]
                    )
                else:
                    nc.gpsimd.collective_compute(
                        "AllGather",
                        ALU.bypass,
                        replica_groups=groups2,
                        ins=[kpv_agin[rep % 2][:]],
                        outs=[kpv_opp[rep % 2][:]],
                    )

            def pass2_chunk(rep, c4):
                """own Qp|V chunk -> QV_t[rep] + qp_tbl[parity]; self logits"""
                if c4 == 0:
                    QV_t[rep] = dbl.tile([128, BLOCKS, 512], F8, tag="QV", name="QV")
                    aself_t[rep] = psS.tile([128, BLOCKS, 2], F32, tag="aself", name="aself")
                QV_res = QV_t[rep]
                aself = aself_t[rep]
                nb4 = min(BLOCKS - c4 * 4, 4)
                sl = slice(c4 * 512, c4 * 512 + nb4 * 128)
                xto = pB.tile([128, 2, 512], F8, tag="xto")
                nc.sync.dma_start(xto[:, :, : nb4 * 128], xTo_d[:, :, sl])
                for bi in range(nb4):
                    b = c4 * 4 + bi
                    bsl = slice(bi * 128, (bi + 1) * 128)
                    ps2 = psB.tile([128, 512], F32, tag="ps2")
                    nc.tensor.matmul(
                        ps2[:], xto[:, 0, bsl], Wqv_sb[:, 0, :],
                        start=True, stop=False,
                    )
                    nc.tensor.matmul(
                        ps2[:], xto[:, 1, bsl], Wqv_sb[:, 1, :],
                        start=False, stop=True,
                    )
                    ps3 = psB.tile([128, 2, 128], F32, tag="ps3")
                    for c in range(2):
                        for f in range(2):
                            nc.tensor.matmul(
                                ps3[:, f, :],
                                Wkpq_sb[:, c, f * 128 : (f + 1) * 128],
                                xto[:, c, bsl],
                                start=(c == 0 and f == 0),
                                stop=(c == 1 and f == 1),
                            )
                    nc.scalar.activation(QV_res[:, b, :], ps2[:], AF.Copy)
                    # self-loop logits: |W2|-folded z^T in ps3 -> signed sum
                    rmx = pB.tile([128, 2, 128], BF, tag="rmx")
                    nc.vector.tensor_scalar_max(rmx[:], ps3[:], 0.0)
                    for h in range(2):
                        nc.tensor.matmul(
                            aself[:, b, h : h + 1],
                            rmx[:, h, :],
                            spm_col[:],
                            start=True,
                            stop=True,
                        )

            def pass2_fin(rep):
                exS_t[rep] = dbl.tile([128, BLOCKS, 2], F32, tag="exS", name="exS")
                nc.scalar.activation(
                    exS_t[rep][:], aself_t[rep][:], AF.Exp,
                    bias=b2f, scale=invg,
                )

            def edge_batch(rep, g):
                QV_res = QV_t[rep]
                exS_res = exS_t[rep]
                nblk = min(BLOCKS - g * BPB, BPB)
                C = nblk * SUBT
                gb = C * 128
                gb16 = gb // 16
                if ABL_GATHER:
                    KpV = KpV_dum
                else:
                    KpV = pG.tile([128, GB // 128, 512], F8, tag="KpV")
                    nc.gpsimd.dma_gather(
                        KpV[:, :C, :],
                        kpv_opp[rep % 2][:],
                        kpix_sb[:, g, :gb16],
                        num_idxs=gb,
                        num_idxs_reg=gb,
                        elem_size=512,
                        transpose=False,
                        single_packet=bool(os.environ.get("SP1")),
                        queue_num=(gat_count[0] % 2)
                        if os.environ.get("SWQ2")
                        else 0,
                    )
                    gat_count[0] += 1
                if ABL_EMATH:
                    ex = ex_dum
                else:
                    # two z tiles alternating by batch parity: each tile has
                    # exactly ONE writer engine (ACT on even batches, DVE on
                    # odd), so PSUM->SBUF copies split across both engines
                    # with no cross-engine write hazards on any tile.
                    z = pW.tile(
                        [128, GB // 128, 256], BF,
                        tag="zE" if g % 2 == 0 else "zO",
                        name="zEO", bufs=1,
                    )
                    for cc in range(C):
                        b_cc = g * BPB + cc // SUBT
                        t_idx = g * BPB * SUBT + cc
                        zq = psQ.tile([128, 256], F32, tag="zq")
                        nc.tensor.matmul(
                            zq[:], ohTq_sb[:, t_idx, :],
                            QV_t[rep][:, b_cc, 0:256],
                            start=True, stop=False,
                        )
                        nc.tensor.matmul(
                            zq[:], ident_sb[:], KpV[:, cc, 0:256],
                            start=False, stop=True,
                        )
                        if g % 2 == 0:
                            nc.scalar.activation(z[:, cc, :], zq[:], AF.Copy)
                        else:
                            nc.vector.tensor_copy(z[:, cc, :], zq[:])
                # signed-W2-folded tables: logit = sum of max-part + min-part
                rzw = pW.tile([128, GB // 128, 2, 128], BF, tag="rzw")
                z4 = z[:, :C, :].rearrange("p c (h j) -> p c h j", h=2)
                if P0 > 0:
                    nc.vector.tensor_scalar_max(
                        rzw[:, :C, :, 0:P0], z4[:, :, :, 0:P0], 0.0
                    )
                if P0 < 128:
                    nc.vector.tensor_scalar_min(
                        rzw[:, :C, :, P0:128], z4[:, :, :, P0:128], 0.0
                    )
                # pairwise tree sum over j (TT adds run 2x); dedicated
                # scratch tiles, written and read only by DVE
                tS = pW.tile([128, GB // 128, 2, 96], BF, tag="tS")
                tA = tS[:, :, :, 0:64]
                tB = tS[:, :, :, 64:96]
                nc.vector.tensor_tensor(
                    tA[:, :C, :, 0:64], rzw[:, :C, :, 0:64],
                    rzw[:, :C, :, 64:128], ALU.add,
                )
                nc.vector.tensor_tensor(
                    tB[:, :C, :, 0:32], tA[:, :C, :, 0:32], tA[:, :C, :, 32:64],
                    ALU.add,
                )
                nc.vector.tensor_tensor(
                    tA[:, :C, :, 0:16], tB[:, :C, :, 0:16], tB[:, :C, :, 16:32],
                    ALU.add,
                )
                nc.vector.tensor_tensor(
                    tB[:, :C, :, 0:8], tA[:, :C, :, 0:8], tA[:, :C, :, 8:16],
                    ALU.add,
                )
                nc.vector.tensor_tensor(
                    tA[:, :C, :, 0:4], tB[:, :C, :, 0:4], tB[:, :C, :, 4:8],
                    ALU.add,
                )
                nc.vector.tensor_tensor(
                    tB[:, :C, :, 0:2], tA[:, :C, :, 0:2], tA[:, :C, :, 2:4],
                    ALU.add,
                )
                ared = pE.tile([128, GB // 128, 2], F32, tag="ared")
                nc.vector.tensor_tensor(
                    ared[:, :C, :],
                    tB[:, :C, :, 0:1].rearrange("p c h o -> p c (h o)"),
                    tB[:, :C, :, 1:2].rearrange("p c h o -> p c (h o)"),
                    ALU.add,
                )
                ex = pE.tile([128, GB // 128, 2], F32, tag="ex")
                nc.scalar.activation(
                    ex[:, :C, :], ared[:, :C, :], AF.Exp, bias=b2f, scale=invg
                )

                if ABL_AGG:
                    nc.sync.dma_start(
                        out_d[:, g * BPB : g * BPB + nblk, :],
                        ost_dum[:, :nblk, :],
                    )
                    return
                for j in range(nblk):
                    b = g * BPB + j
                    aggs = psE.tile([128, EMB + 2], F32, tag="aggs", bufs=1)
                    first = True
                    for s in range(SUBT):
                        cc = j * SUBT + s
                        t_idx = b * SUBT + s
                        for h in range(2):
                            oh = pE.tile([128, 128], BF, tag="oh")
                            nc.vector.tensor_scalar(
                                oh[:],
                                iota_bc[:],
                                dstrel_sb[:, t_idx : t_idx + 1],
                                ex[:, cc, h : h + 1],
                                op0=ALU.is_equal,
                                op1=ALU.mult,
                            )
                            nc.tensor.matmul(
                                aggs[:, h * 128 : (h + 1) * 128],
                                KpV[:, cc, 256 + h * 128 : 256 + (h + 1) * 128],
                                oh[:],
                                start=first,
                                stop=False,
                            )
                            first = False
                            nc.tensor.matmul(
                                aggs[:, EMB + h : EMB + h + 1],
                                oh[:],
                                ones_col[:],
                                start=False,
                                stop=False,
                            )
                    for h in range(2):
                        dg = pE.tile([128, 128], BF, tag="dg")
                        nc.vector.tensor_scalar(
                            dg[:],
                            iota_bc[:],
                            iota_col[:],
                            exS_res[:, b, h : h + 1],
                            op0=ALU.is_equal,
                            op1=ALU.mult,
                        )
                        nc.tensor.matmul(
                            aggs[:, h * 128 : (h + 1) * 128],
                            QV_res[:, b, 256 + h * 128 : 256 + (h + 1) * 128],
                            dg[:],
                            start=False,
                            stop=False,
                        )
                        nc.tensor.matmul(
                            aggs[:, EMB + h : EMB + h + 1],
                            dg[:],
                            ones_col[:],
                            start=False,
                            stop=(h == 1),
                        )
                    r = pE.tile([128, 2], F32, tag="r")
                    nc.vector.reciprocal(r[:], aggs[:, EMB : EMB + 2])
                    ag = pE.tile([128, 256], BF, tag="ag")
                    nc.scalar.activation(ag[:], aggs[:, 0:256], AF.Copy)
                    P = psE.tile([128, 2, EMB], F32, tag="P", bufs=1)
                    nc.tensor.matmul(
                        P[:, 0, :], ag[:, 0:128], Wout_sb[:, 0, :],
                        start=True, stop=True,
                    )
                    nc.tensor.matmul(
                        P[:, 1, :], ag[:, 128:256], Wout_sb[:, 1, :],
                        start=True, stop=True,
                    )
                    t0 = pE.tile([128, EMB], BF, tag="t0")
                    nc.scalar.activation(
                        t0[:], P[:, 0, :], AF.Copy, scale=r[:, 0:1]
                    )
                    t1 = pE.tile([128, EMB], BF, tag="t1")
                    nc.scalar.activation(
                        t1[:], P[:, 1, :], AF.Copy, scale=r[:, 1:2]
                    )
                    u = pE.tile([128, EMB], BF, tag="u")
                    nc.vector.tensor_tensor(u[:], t0[:], t1[:], ALU.add)
                    rl = pE.tile([128, EMB], BF, tag="rl")
                    nc.vector.tensor_scalar_max(rl[:], u[:], 0.0)
                    if j == 0:
                        ost = pE.tile([128, BPB, EMB], BF, tag="ost", name="ost")
                    nc.vector.tensor_tensor(
                        ost[:, j, :], rl[:], x_res[:, b, :], ALU.add
                    )
                nc.sync.dma_start(
                    out_d[:, g * BPB : g * BPB + nblk, :],
                    ost[:, :nblk, :],
                )

            # ---- schedule: software-pipeline phase A of rep k+1 into the
            # edge phase of rep k, hiding the AllGather behind edge work.
            g_p1end = max(1, min((NB4 + 1) // 2, NB // 2))
            p1_per = (NB4 + g_p1end - 1) // g_p1end
            g_ag = int(os.environ.get("AG_AT", str(min(NB - 1, g_p1end))))
            g_p2 = min(g_p1end + 1, NB - 1)  # pass2 start (independent of AG)
            n_p2_slots = max(1, NB - g_p2)
            p2_per = (NB4 + n_p2_slots - 1) // n_p2_slots

            def phaseA_sched(g):
                """(pass1 chunks, do_collective, pass2 chunks) at edge batch g."""
                p1 = (
                    list(range(g * p1_per, min((g + 1) * p1_per, NB4)))
                    if g < g_p1end
                    else []
                )
                p2 = []
                if g >= g_p2:
                    k = g - g_p2
                    p2 = list(range(k * p2_per, min((k + 1) * p2_per, NB4)))
                return p1, g == g_ag, p2

            # prologue: phase A of rep 0
            for c4 in range(NB4):
                pass1_chunk(0, c4)
            collective(0)
            for c4 in range(NB4):
                pass2_chunk(0, c4)
            pass2_fin(0)

            for rep in range(repeats):
                nxt = rep + 1
                for g in range(NB):
                    edge_batch(rep, g)
                    if nxt < repeats:
                        p1, do_ag, p2 = phaseA_sched(g)
                        for c4 in p1:
                            pass1_chunk(nxt, c4)
                        if do_ag:
                            collective(nxt)
                        for c4 in p2:
                            pass2_chunk(nxt, c4)
                        if g == NB - 1:
                            pass2_fin(nxt)
                # free stale per-rep handles
                QV_t.pop(rep - 1, None)
                exS_t.pop(rep - 1, None)
                aself_t.pop(rep - 1, None)

        for f in _frees:
            f()

    nc.compile()
    return nc


# ------------------------------------------------------------------ driver


def assemble_out(oc, node_of_c, N, EMB, out):
    """Scatter one core's [128, BLOCKS, EMB] output into the full array."""
    blocks = oc.shape[1]
    flat = np.asarray(oc).astype(np.float32).transpose(1, 0, 2).reshape(-1, EMB)
    valid = node_of_c >= 0
    out[node_of_c[valid]] = flat[valid]


def _build_all(inputs, n_cores=8, repeats=1):
    meta, per_core, consts, node_of = prep(n_cores=n_cores, **inputs)
    nc = build_program(meta, repeats=repeats)
    in_maps = []
    for c in range(n_cores):
        m = dict(per_core[c])
        m.update(consts)
        in_maps.append({k: np.ascontiguousarray(v) for k, v in m.items()})
    return meta, nc, in_maps, node_of


def kernel(**inputs):
    import concourse.bass_utils as bass_utils

    n_cores = 8
    meta, nc, in_maps, node_of = _build_all(inputs, n_cores)
    res = bass_utils.run_bass_kernel_spmd(
        nc, in_maps, core_ids=list(range(n_cores))
    )
    N, EMB = inputs["x"].shape
    out = np.zeros((N, EMB), np.float32)
    for c in range(n_cores):
        assemble_out(res.results[c]["out"], node_of[c], N, EMB, out)
    return out

